# revision 9
# baseline (speedup 1.0000x reference)
"""Causal self-attention (GQA + RoPE + QK-norm) Trainium2 Bass kernel.

Sharding: 8 cores = 4 batches x 2 head-groups.  Core c -> batch c//2,
q heads (c%2)*8..+8, kv heads (c%2)*2..+2.  wproj is row-sharded, so each
core emits a partial (T, C) output in bf16; the host upcasts and sums the
two partials per batch.

Device-side layout strategy (per core):
  - x is fed pre-transposed (xT, [C, T]) and bf16-cast by the host; x tiles
    are DMA'd in 256-token pairs so each descriptor is >=512B (half the
    per-byte DMA cost of 256B descriptors).
  - QKV projections produce Q,K token-major ([tok, cols]); RoPE runs
    token-major on DVE while the rms-norm sums run on the scalar engine
    directly from the pre-RoPE PSUM (RoPE is a rotation, so it preserves
    per-head norms); 128x128 PE transposes
    produce qT/kT feature-major for the attention matmuls.  V is produced
    token-major, which is exactly the p@v stationary layout.
  - scores are computed transposed (scoresT[tk, tq]) so that after exp the
    p tiles are already the moving operand for the p@v matmul; the softmax
    denominator comes from ones-column matmuls accumulated in PSUM: chunk
    pairs away from the diagonal are cast to fp8e5m2 and use a DoubleRow
    matmul (half cost; quantization noise averages out in the row sum),
    while diagonal chunks stay bf16.  Denominator matmuls are emitted two
    chunks late so the next head's accumulation never waits on the
    reciprocal chain.  The reciprocal is broadcast across partitions on
    the gpsimd engine.
  - exp has no max-subtraction: qk-norm bounds |s| <= sqrt(128) ~ 11.32.
  - output projection units of tile t-1 are interleaved between the
    attention heads of tile t so they fill softmax pipeline bubbles;
    partials are cast to bf16 on the scalar engine and DMA'd out.
"""

import numpy as np
import ml_dtypes
from collections import deque
from contextlib import ExitStack

import concourse.bass as bass
import concourse.mybir as mybir
import concourse.tile as tile
from concourse import bacc
from concourse.bass_utils import run_bass_kernel_spmd
from concourse.masks import make_identity

BF16 = mybir.dt.bfloat16
F32 = mybir.dt.float32
F8 = mybir.dt.float8e5
AF = mybir.ActivationFunctionType

B, T, C = 4, 2048, 2048
H, KV, D = 16, 4, 128
HG, KVG = H // 2, KV // 2          # per-core q heads (8), kv heads (2)
QC, KC = HG * D, KVG * D           # 1024, 256
P = 128
TOKCH = T // P                     # 16 token chunks
NREP = H // KV                     # 4
EPS = 1e-5
NEG = -1.0e5                       # additive causal mask (exp -> 0)


def _build():
    nc = bacc.Bacc("TRN2", target_bir_lowering=False, debug=False, num_devices=8)
    xt = nc.dram_tensor("xt", [C, T], BF16, kind="ExternalInput")
    wq = nc.dram_tensor("wq", [C, QC], BF16, kind="ExternalInput")
    wkv = nc.dram_tensor("wkv", [C, 2 * KC], BF16, kind="ExternalInput")
    wp = nc.dram_tensor("wp", [QC, C], BF16, kind="ExternalInput")
    csd = nc.dram_tensor("csd", [T, D], F32, kind="ExternalInput")  # cos||sin
    out = nc.dram_tensor("out", [T, C], BF16, kind="ExternalOutput")

    with tile.TileContext(nc) as tc, ExitStack() as ctx:
        singles = ctx.enter_context(tc.tile_pool(name="singles", bufs=1))
        actx = ExitStack()
        xpool = actx.enter_context(tc.tile_pool(name="xa", bufs=2))

        # ---- prefetch the first two x pairs on the gpsimd DMA queue ----
        xr = xt.rearrange("(co p) t -> p co t", p=P)
        xpairs = {}
        wq_sb = singles.tile([P, C // P, QC], BF16)
        wkv_sb = singles.tile([P, C // P, 2 * KC], BF16)
        wqr = wq.rearrange("(co p) q -> p co q", p=P)
        wkvr = wkv.rearrange("(co p) q -> p co q", p=P)
        cs_sb = singles.tile([P, TOKCH, D], F32)   # [:, :, 0:64]=cos, 64:128=sin

        xp0 = xpool.tile([P, C // P, 2 * P], BF16, tag="xt")
        xp1 = xpool.tile([P, C // P, 2 * P], BF16, tag="xt")
        # arrival-ordered: feed chunk 0's first matmuls ASAP, then weights in
        # co order interleaved with the rest of the early x pairs
        nc.sync.dma_start(xp0[:, 0:4, :], xr[:, 0:4, 0:2 * P])
        nc.sync.dma_start(wq_sb[:, 0:4, :], wqr[:, 0:4, :])
        nc.sync.dma_start(wkv_sb[:, 0:4, :], wkvr[:, 0:4, :])
        nc.sync.dma_start(xp0[:, 4:16, :], xr[:, 4:16, 0:2 * P])
        nc.sync.dma_start(wq_sb[:, 4:8, :], wqr[:, 4:8, :])
        nc.sync.dma_start(wkv_sb[:, 4:8, :], wkvr[:, 4:8, :])
        nc.sync.dma_start(cs_sb, csd.rearrange("(tc p) d -> p tc d", p=P))
        nc.sync.dma_start(xp1, xr[:, :, 2 * P:4 * P])
        for gco in range(2, 4):
            sl = slice(4 * gco, 4 * gco + 4)
            nc.sync.dma_start(wq_sb[:, sl, :], wqr[:, sl, :])
            nc.sync.dma_start(wkv_sb[:, sl, :], wkvr[:, sl, :])
        xpairs[0] = xp0
        xpairs[1] = xp1

        ident = singles.tile([P, P], BF16)
        make_identity(nc, ident)
        ones_col = singles.tile([P, 1], BF16)
        nc.vector.memset(ones_col, 1.0)
        ones_f8 = singles.tile([P, 2, 16], F8)     # DoubleRow stationary ones
        nc.vector.memset(ones_f8, 1.0)
        zero_col = singles.tile([P, 1], F32)
        nc.vector.memset(zero_col, 0.0)
        eps_col = singles.tile([P, 1], F32)
        nc.vector.memset(eps_col, EPS)
        epsq_col = singles.tile([P, 1], F32)
        nc.vector.memset(epsq_col, float(D * EPS))
        nc.const_aps.aps[(F32, 0.0)] = zero_col[:]
        nc.const_aps.aps[(F32, EPS)] = eps_col[:]
        nc.const_aps.aps[(F32, float(D * EPS))] = epsq_col[:]

        # 4 diagonal-block masks: variant o (offset o*128): keep where
        # i >= j + o*128  (j = tk partition, i = tq free)
        mask_sb = singles.tile([P, 4, 512], F32)
        nc.vector.memset(mask_sb, 0.0)
        for o in range(4):
            nc.gpsimd.affine_select(
                out=mask_sb[:, o, :], in_=mask_sb[:, o, :],
                compare_op=mybir.AluOpType.is_ge, fill=NEG,
                base=-o * P, pattern=[[1, 512]], channel_multiplier=-1,
            )

        qT = singles.tile([P, HG, T], BF16)      # [d, h, tok]
        kT = singles.tile([P, KVG, T], BF16)
        v_sb = singles.tile([P, TOKCH, KC], BF16)  # [tok%128, chunk, vcol]
        yT = singles.tile([P, HG, T], BF16)
        scr = singles.tile([P, D], F32)          # Square-activation sink

        # ================= phase A: QKV proj + RoPE + qk-norm =============
        with tc.tile_pool(name="pa", bufs=2, space="PSUM") as pps, \
             tc.tile_pool(name="sa", bufs=3) as spool:
            for t in range(TOKCH):
                pr, half = t // 2, t % 2
                if half == 0:
                    if pr in xpairs:
                        xpair = xpairs.pop(pr)
                    else:
                        xpair = xpool.tile([P, C // P, 2 * P], BF16, tag="xt")
                        nc.sync.dma_start(xpair, xr[:, :, 2 * pr * P:(2 * pr + 2) * P])
                    cur_pair = xpair
                xtile = cur_pair[:, :, half * P:(half + 1) * P]
                ps_q0 = pps.tile([P, 512], F32, tag="q0")
                ps_q1 = pps.tile([P, 512], F32, tag="q1")
                ps_kv = pps.tile([P, 512], F32, tag="kv")
                ps_k = ps_kv[:, 0:KC]
                ps_v = ps_kv[:, KC:2 * KC]
                nco = C // P
                for co in range(nco):
                    lhsT = xtile[:, co, :]
                    st = dict(start=(co == 0), stop=(co == nco - 1))
                    nc.tensor.matmul(ps_q0, lhsT, wq_sb[:, co, 0:512], **st)
                    nc.tensor.matmul(ps_q1, lhsT, wq_sb[:, co, 512:1024], **st)
                    nc.tensor.matmul(ps_kv, lhsT, wkv_sb[:, co, :], **st)

                # V: cast straight to resident token-major buffer
                nc.vector.tensor_copy(v_sb[:, t, :], ps_v)

                # Q/K: rms-norm sums on ACT from pre-rope PSUM (rope is a
                # rotation, so it preserves per-head norms); rope on DVE;
                # PE transposes to feature-major.
                def rope_norm(ps, nh, dstT, h0, is_q):
                    h2 = D // 2
                    ss = spool.tile([P, nh], F32, tag=f"ss{nh}")
                    for i in range(nh):
                        nc.scalar.activation(scr, ps[:, i * D:(i + 1) * D],
                                             AF.Square, accum_out=ss[:, i:i + 1])
                    # q: rq = 1/sqrt(ss + D*eps)  (folds in the 1/sqrt(D)
                    # attention scale); k: rq = 1/sqrt(ss/D + eps)
                    rt = spool.tile([P, nh], F32, tag=f"rt{nh}")
                    if is_q:
                        nc.scalar.activation(rt, ss, AF.Sqrt, scale=1.0,
                                             bias=float(D * EPS))
                    else:
                        nc.scalar.activation(rt, ss, AF.Sqrt, scale=1.0 / D,
                                             bias=EPS)
                    rq = spool.tile([P, nh], F32, tag=f"rq{nh}")
                    nc.vector.reciprocal(rq, rt)

                    v4 = ps.rearrange("p (h a d) -> p h a d", h=nh, a=2)
                    q1, q2 = v4[:, :, 0, :], v4[:, :, 1, :]
                    r = spool.tile([P, nh, 2, h2], F32, tag=f"rope{nh}")
                    r1, r2 = r[:, :, 0, :], r[:, :, 1, :]
                    s2 = spool.tile([P, nh, h2], F32, tag=f"scr{nh}")
                    cs = cs_sb[:, t, None, 0:h2].to_broadcast([P, nh, h2])
                    sn = cs_sb[:, t, None, h2:D].to_broadcast([P, nh, h2])
                    nc.vector.tensor_mul(r1, q1, cs)
                    nc.vector.tensor_mul(s2, q2, sn)
                    nc.vector.tensor_sub(r1, r1, s2)
                    nc.vector.tensor_mul(r2, q1, sn)
                    nc.vector.tensor_mul(s2, q2, cs)
                    nc.vector.tensor_add(r2, r2, s2)
                    rf = r.rearrange("p h a d -> p h (a d)")
                    qbf = spool.tile([P, nh, D], BF16, tag=f"qbf{nh}")
                    nc.vector.tensor_mul(qbf, rf, rq[:, :, None].to_broadcast([P, nh, D]))
                    pst = pps.tile([P, 4, P], BF16, tag="tr")
                    for i in range(nh):
                        nc.tensor.transpose(pst[:, i, :], qbf[:, i, :], ident)
                    # one strided copy: psum [128, nh*128] -> nh head slices
                    nc.scalar.copy(
                        dstT[:, h0:h0 + nh, t * P:(t + 1) * P], pst[:, 0:nh, :])

                rope_norm(ps_q0, 4, qT, 0, True)
                rope_norm(ps_q1, 4, qT, 4, True)
                rope_norm(ps_k, KVG, kT, 0, False)
        actx.close()  # release the x pool before the B/C pools open

        # ======== phase B+C: attention + output projection, interleaved ====
        with tc.tile_pool(name="wpp", bufs=1) as wpool, \
             tc.tile_pool(name="psc", bufs=3, space="PSUM") as psc, \
             tc.tile_pool(name="psy", bufs=2, space="PSUM") as psy, \
             tc.tile_pool(name="pss", bufs=1, space="PSUM") as pss, \
             tc.tile_pool(name="pso", bufs=2, space="PSUM") as pso, \
             tc.tile_pool(name="pb", bufs=4) as ppool, \
             tc.tile_pool(name="pq", bufs=2) as qpool, \
             tc.tile_pool(name="sb", bufs=3) as bpool, \
             tc.tile_pool(name="so", bufs=6) as opool:
            wpr = wp.rearrange("(hc p) c -> p hc c", p=P)
            wp_ts = []
            for ct in range(C // 512):
                wp_t = wpool.tile([P, HG, 512], BF16, tag=f"wpt{ct}")
                nc.sync.dma_start(wp_t, wpr[:, :, ct * 512:(ct + 1) * 512])
                wp_ts.append(wp_t)

            def proj_unit(tk, ct):
                ps_o = pso.tile([P, 512], F32, tag="o")
                for hc in range(HG):
                    nc.tensor.matmul(
                        ps_o, yT[:, hc, tk * P:(tk + 1) * P],
                        wp_ts[ct][:, hc, :],
                        start=(hc == 0), stop=(hc == HG - 1))
                ob = opool.tile([P, 512], BF16, tag="ob")
                nc.vector.tensor_copy(ob, ps_o)
                nc.sync.dma_start(
                    out[tk * P:(tk + 1) * P, ct * 512:(ct + 1) * 512], ob)

            cqueue = deque()
            NT = T // 512  # 4 tq tiles
            # big tiles first: small-tile heads then have projection units of
            # the previous tile available to fill pipeline bubbles
            for t in (3, 2, 1, 0):
                for h in range(HG):
                    g = h // NREP
                    nch = 4 * (t + 1)
                    n_clean = 4 * t       # chunks in fp8 DoubleRow den pairs
                    ps_y = psy.tile([P, 512], F32, tag="y")
                    ps_sden = pss.tile([P, 512], F32, tag="sden")
                    ps_s = ps_sden[0:1, :]

                    # den ops in accumulation order, each with the chunk index
                    # after whose processing its inputs are ready
                    den_plan = deque()
                    n_den_total = n_clean // 2 + (nch - n_clean)
                    den_idx = [0]
                    ptqs = {}

                    def emit_den(op_kind, arg):
                        first = den_idx[0] == 0
                        last = den_idx[0] == n_den_total - 1
                        den_idx[0] += 1
                        st = dict(start=first, stop=last)
                        if op_kind == "dr":
                            nc.tensor.matmul(
                                ps_s, ones_f8[:, :, 0:1], ptqs[arg],
                                perf_mode=mybir.MatmulPerfMode.DoubleRow,
                                skip_group_check=True, **st)
                        else:
                            c, col0, pt = arg
                            nc.tensor.matmul(ps_s[:, col0:512], ones_col,
                                             pt[:, col0:512],
                                             skip_group_check=True, **st)

                    for c in range(nch):
                        o = c * P - t * 512
                        col0 = max(o, 0)
                        ps_sc = psc.tile([P, 512], F32, tag="sc")
                        nc.tensor.matmul(
                            ps_sc[:, col0:512], kT[:, g, c * P:(c + 1) * P],
                            qT[:, h, t * 512 + col0:(t + 1) * 512],
                            start=True, stop=True)
                        if o >= 0:
                            # after the col0 shift the partial block is always
                            # the i' >= j triangle
                            nc.vector.tensor_add(ps_sc[:, col0:col0 + P],
                                                 ps_sc[:, col0:col0 + P],
                                                 mask_sb[:, 0, 0:P])
                        pt = ppool.tile([P, 512], BF16, tag="pt")
                        nc.scalar.activation(pt[:, col0:512], ps_sc[:, col0:512], AF.Exp)
                        st = dict(start=(c == 0), stop=(c == nch - 1))
                        nc.tensor.matmul(ps_y[:, col0:512],
                                         v_sb[:, c, g * P:(g + 1) * P],
                                         pt[:, col0:512], **st)
                        if c < n_clean:
                            pr, hf = c // 2, c % 2
                            if hf == 0:
                                ptqs[pr] = qpool.tile([P, 2, 512], F8, tag="ptq", name="ptq")
                            # alternate cast engine to split the load
                            if hf == 0:
                                nc.vector.tensor_copy(ptqs[pr][:, 0, :], pt)
                            else:
                                nc.gpsimd.tensor_copy(ptqs[pr][:, 1, :], pt)
                                den_plan.append((2 * pr + 1, "dr", pr))
                        else:
                            den_plan.append((c, "bf", (c, col0, pt)))
                        # emit dens whose inputs were ready >= 2 chunks ago
                        while den_plan and den_plan[0][0] <= c - 2:
                            _, kind, arg = den_plan.popleft()
                            emit_den(kind, arg)
                    while den_plan:
                        _, kind, arg = den_plan.popleft()
                        emit_den(kind, arg)

                    rc = bpool.tile([1, 512], F32, tag="rc")
                    nc.vector.reciprocal(rc, ps_s)
                    rb = bpool.tile([P, 512], F32, tag="rb")
                    nc.gpsimd.partition_broadcast(rb, rc)
                    nc.vector.tensor_mul(yT[:, h, t * 512:(t + 1) * 512], ps_y, rb)

                    # fill pipeline bubbles with projection units of tile t-1
                    for _ in range(2):
                        if cqueue:
                            proj_unit(*cqueue.popleft())

                for tci in range(4):
                    for ct in range(C // 512):
                        cqueue.append((4 * t + tci, ct))
            while cqueue:
                proj_unit(*cqueue.popleft())
    nc.compile()
    return nc


_NC_CACHE = []


def _get_prog():
    if not _NC_CACHE:
        _NC_CACHE.append(_build())
    return _NC_CACHE[0]


def _make_in_maps(inputs):
    x, cos, sin = inputs["x"], inputs["cos"], inputs["sin"]
    wq, wk, wv, wproj = inputs["wq"], inputs["wk"], inputs["wv"], inputs["wproj"]
    bf = ml_dtypes.bfloat16
    cos2 = np.ascontiguousarray(cos.reshape(T, D // 2), dtype=np.float32)
    sin2 = np.ascontiguousarray(sin.reshape(T, D // 2), dtype=np.float32)
    cs2 = np.ascontiguousarray(np.hstack([cos2, sin2]))
    in_maps = []
    for core in range(8):
        b, g = core // 2, core % 2
        qs = slice(g * QC, (g + 1) * QC)
        ks = slice(g * KC, (g + 1) * KC)
        in_maps.append({
            "xt": np.ascontiguousarray(x[b].T).astype(bf),
            "wq": np.ascontiguousarray(wq[:, qs]).astype(bf),
            "wkv": np.ascontiguousarray(np.hstack([wk[:, ks], wv[:, ks]])).astype(bf),
            "wp": np.ascontiguousarray(wproj[qs, :]).astype(bf),
            "csd": cs2,
        })
    return in_maps


def kernel(x, cos, sin, wq, wk, wv, wproj):
    nc = _get_prog()
    in_maps = _make_in_maps(dict(x=x, cos=cos, sin=sin, wq=wq, wk=wk, wv=wv, wproj=wproj))
    res = run_bass_kernel_spmd(nc, in_maps, core_ids=list(range(8))).results
    outp = np.empty((B, T, C), np.float32)
    for b in range(B):
        outp[b] = (res[2 * b]["out"].astype(np.float32)
                   + res[2 * b + 1]["out"].astype(np.float32))
    return outp


# revision 31
# speedup vs baseline: 1.0899x; 1.0899x over previous
"""Causal self-attention (GQA + RoPE + QK-norm) Trainium2 Bass kernel.

Sharding: 8 cores = 4 batches x 2 head-groups.  Core c -> batch c//2,
q heads (c%2)*8..+8, kv heads (c%2)*2..+2.  wproj is row-sharded, so each
core emits a partial (T, C) output in bf16; the host upcasts and sums the
two partials per batch.

Device-side layout strategy (per core):
  - x is fed pre-transposed (xT, [C, T]) and bf16-cast by the host; x is
    DMA'd in 256-token pairs (>=512B descriptors halve the per-byte DMA
    cost), with the first pair split per 4-co quarter and interleaved with
    the weight co-groups so the first QKV matmuls start as soon as possible.
  - QKV projections produce Q,K token-major; RoPE runs token-major on DVE
    while the rms-norm sums run on the scalar engine directly from the
    pre-RoPE PSUM (RoPE is a rotation, so it preserves per-head norms);
    128x128 PE transposes produce qT/kT feature-major for the attention
    matmuls.  V lands token-major, exactly the p@v stationary layout.
  - a single PSUM scope is shared by all phases (phase A accumulators alias
    the attention-phase pools) so the tile scheduler can overlap the phase
    boundary; the RoPE of the last two token chunks is deferred and
    interleaved with pre-emitted tile-0 attention scores so the PE stays
    busy through phase A's serial tail.
  - scores are computed transposed (scoresT[tk, tq]) so after exp the p
    tiles are already the moving operand for p@v; the softmax denominator
    comes from ones-column matmuls accumulated in PSUM: chunk pairs away
    from the diagonal are cast to fp8e5m2 and use a DoubleRow matmul (half
    cost; quantization noise averages out in the row sum), diagonal chunks
    stay bf16.  Denominator matmuls are emitted two chunks late so the next
    head's accumulation never waits on the reciprocal chain, whose
    reciprocal is broadcast across partitions on the gpsimd engine.
  - exp has no max-subtraction: qk-norm bounds |s| <= sqrt(128) ~ 11.32.
  - output projection units of tile t-1 are interleaved between the
    attention heads of tile t to fill softmax pipeline bubbles; partials
    are cast to bf16 on the scalar engine and DMA'd out.
"""

import os
import numpy as np
import ml_dtypes
from collections import deque
from contextlib import ExitStack

import concourse.bass as bass
import concourse.mybir as mybir
import concourse.tile as tile
from concourse import bacc
from concourse.bass_utils import run_bass_kernel_spmd

BF16 = mybir.dt.bfloat16
F32 = mybir.dt.float32
F8 = mybir.dt.float8e5
AF = mybir.ActivationFunctionType

B, T, C = 4, 2048, 2048
H, KV, D = 16, 4, 128
HG, KVG = H // 2, KV // 2          # per-core q heads (8), kv heads (2)
QC, KC = HG * D, KVG * D           # 1024, 256
P = 128
TOKCH = T // P                     # 16 token chunks
NREP = H // KV                     # 4
EPS = 1e-5
NEG = -1.0e5                       # additive causal mask (exp -> 0)

DEN_DELAY = int(os.environ.get("DEN_DELAY", "2"))
PRE_HEADS = int(os.environ.get("PRE_HEADS", "2"))
DEFER_CHUNKS = int(os.environ.get("DEFER_CHUNKS", "2"))


def _build():
    nc = bacc.Bacc("TRN2", target_bir_lowering=False, debug=False, num_devices=8)
    xt = nc.dram_tensor("xt", [C, T], BF16, kind="ExternalInput")
    wq = nc.dram_tensor("wq", [C, QC], BF16, kind="ExternalInput")
    wkv = nc.dram_tensor("wkv", [C, 2 * KC], BF16, kind="ExternalInput")
    wp = nc.dram_tensor("wp", [QC, C], BF16, kind="ExternalInput")
    csd = nc.dram_tensor("csd", [T, D], F32, kind="ExternalInput")  # cos||sin
    out = nc.dram_tensor("out", [T, C], BF16, kind="ExternalOutput")

    with tile.TileContext(nc) as tc, ExitStack() as ctx:
        singles = ctx.enter_context(tc.tile_pool(name="singles", bufs=1))

        xr = xt.rearrange("(co p) t -> p co t", p=P)
        wq_sb = singles.tile([P, C // P, QC], BF16)
        wkv_sb = singles.tile([P, C // P, 2 * KC], BF16)
        wqr = wq.rearrange("(co p) q -> p co q", p=P)
        wkvr = wkv.rearrange("(co p) q -> p co q", p=P)
        cs_sb = singles.tile([P, TOKCH, D], F32)   # [:, :, 0:64]=cos, 64:128=sin

        ones_col = singles.tile([P, 1], BF16)
        nc.vector.memset(ones_col, 1.0)
        ones_f8 = singles.tile([P, 2, 16], F8)     # DoubleRow stationary ones
        nc.vector.memset(ones_f8, 1.0)
        zero_col = singles.tile([P, 1], F32)
        nc.vector.memset(zero_col, 0.0)
        eps_col = singles.tile([P, 1], F32)
        nc.vector.memset(eps_col, EPS)
        epsq_col = singles.tile([P, 1], F32)
        nc.vector.memset(epsq_col, float(D * EPS))
        nc.const_aps.aps[(F32, 0.0)] = zero_col[:]
        nc.const_aps.aps[(F32, EPS)] = eps_col[:]
        nc.const_aps.aps[(F32, float(D * EPS))] = epsq_col[:]

        # 4 diagonal-block masks: variant o (offset o*128): keep where
        # i >= j + o*128  (j = tk partition, i = tq free)
        mask_sb = singles.tile([P, 4, 512], F32)
        nc.vector.memset(mask_sb, 0.0)
        for o in range(4):
            nc.gpsimd.affine_select(
                out=mask_sb[:, o, :], in_=mask_sb[:, o, :],
                compare_op=mybir.AluOpType.is_ge, fill=NEG,
                base=-o * P, pattern=[[1, 512]], channel_multiplier=-1,
            )

        qT = singles.tile([P, HG, T], BF16)      # [d, h, tok]
        kT = singles.tile([P, KVG, T], BF16)
        v_sb = singles.tile([P, TOKCH, KC], BF16)  # [tok%128, chunk, vcol]
        yT = singles.tile([P, HG, T], BF16)
        scr = singles.tile([P, D], F32)          # Square-activation sink
        gate = singles.tile([P, 1], F32)         # zero, data-dependent on the
        # last phase-A Sqrt: used as every exp's bias so the scheduled Act
        # stream has all Sqrts before all Exps (exactly one table switch)

        # ======= unified PSUM scope: phase A accumulators alias B/C pools ===
        psum_ctx = ExitStack()
        psc = psum_ctx.enter_context(tc.tile_pool(name="psc", bufs=3, space="PSUM"))
        psy = psum_ctx.enter_context(tc.tile_pool(name="psy", bufs=2, space="PSUM"))
        pso = psum_ctx.enter_context(tc.tile_pool(name="pso", bufs=2, space="PSUM"))
        pss = psum_ctx.enter_context(tc.tile_pool(name="pss", bufs=1, space="PSUM"))

        preB = {}     # h -> [pt tiles for tile-0 chunks]
        ppool_ctx = ExitStack()
        ppool = ppool_ctx.enter_context(tc.tile_pool(name="pb", bufs=6))

        # ---- x prefetch, arrival-ordered with the weight co-groups ----
        actx = ExitStack()
        xpool = actx.enter_context(tc.tile_pool(name="xa", bufs=2))
        xp0 = xpool.tile([P, C // P, 2 * P], BF16, tag="xt")
        xp1 = xpool.tile([P, C // P, 2 * P], BF16, tag="xt")
        xpairs = {}
        nc.sync.dma_start(xp0[:, 0:4, :], xr[:, 0:4, 0:2 * P])
        nc.sync.dma_start(wq_sb[:, 0:4, :], wqr[:, 0:4, :])
        nc.sync.dma_start(wkv_sb[:, 0:4, :], wkvr[:, 0:4, :])
        nc.sync.dma_start(xp0[:, 4:16, :], xr[:, 4:16, 0:2 * P])
        nc.sync.dma_start(wq_sb[:, 4:8, :], wqr[:, 4:8, :])
        nc.sync.dma_start(wkv_sb[:, 4:8, :], wkvr[:, 4:8, :])
        nc.sync.dma_start(cs_sb, csd.rearrange("(tc p) d -> p tc d", p=P))
        nc.sync.dma_start(xp1, xr[:, :, 2 * P:4 * P])
        for gco in range(2, 4):
            sl = slice(4 * gco, 4 * gco + 4)
            nc.sync.dma_start(wq_sb[:, sl, :], wqr[:, sl, :])
            nc.sync.dma_start(wkv_sb[:, sl, :], wkvr[:, sl, :])
        xpairs[0] = xp0
        xpairs[1] = xp1

        # ================= phase A: QKV proj + RoPE + qk-norm =============
        with tc.tile_pool(name="sa", bufs=3) as spool:

            def norm_part(ps, nh, is_q):
                # rms-norm sums from the pre-rope PSUM; the attention
                # 1/sqrt(D) scale folds into the q variant:
                # q: rq = 1/sqrt(ss + D*eps); k: rq = 1/sqrt(ss/D + eps)
                ss = spool.tile([P, nh], F32, tag=f"ss{nh}", bufs=6)
                for i in range(nh):
                    nc.scalar.activation(scr, ps[:, i * D:(i + 1) * D],
                                         AF.Square, accum_out=ss[:, i:i + 1])
                rt = spool.tile([P, nh], F32, tag=f"rt{nh}", bufs=6)
                if is_q:
                    nc.scalar.activation(rt, ss, AF.Sqrt, scale=1.0,
                                         bias=float(D * EPS))
                else:
                    nc.scalar.activation(rt, ss, AF.Sqrt, scale=1.0 / D,
                                         bias=EPS)
                rq = spool.tile([P, nh], F32, tag=f"rq{nh}", bufs=6)
                nc.vector.reciprocal(rq, rt)
                return rq

            def rope_part(ps, nh, dstT, h0, t, rq):
                h2 = D // 2
                v4 = ps.rearrange("p (h a d) -> p h a d", h=nh, a=2)
                q1, q2 = v4[:, :, 0, :], v4[:, :, 1, :]
                r = spool.tile([P, nh, 2, h2], F32, tag=f"rope{nh}")
                r1, r2 = r[:, :, 0, :], r[:, :, 1, :]
                s2 = spool.tile([P, nh, h2], F32, tag=f"scr{nh}")
                cs = cs_sb[:, t, None, 0:h2].to_broadcast([P, nh, h2])
                sn = cs_sb[:, t, None, h2:D].to_broadcast([P, nh, h2])
                nc.vector.tensor_mul(r1, q1, cs)
                nc.vector.tensor_mul(s2, q2, sn)
                nc.vector.tensor_sub(r1, r1, s2)
                nc.vector.tensor_mul(r2, q1, sn)
                nc.vector.tensor_mul(s2, q2, cs)
                nc.vector.tensor_add(r2, r2, s2)
                rf = r.rearrange("p h a d -> p h (a d)")
                qbf = spool.tile([P, nh, D], BF16, tag=f"qbf{nh}")
                nc.vector.tensor_mul(qbf, rf, rq[:, :, None].to_broadcast([P, nh, D]))
                # feature-major via the DMA xbar transpose: off the PE/Act
                # critical path entirely (32x32 tiles, 14ns/tile model cost)
                for i in range(nh):
                    nc.sync.dma_start_transpose(
                        dstT[:, h0 + i, t * P:(t + 1) * P], qbf[:, i, :])

            pending = []   # deferred rope work for the last chunks
            for t in range(TOKCH):
                pr, half = t // 2, t % 2
                if half == 0:
                    cur_pair = xpairs.pop(pr)
                ps_q0 = psy.tile([P, 512], F32, tag="y")
                ps_q1 = pso.tile([P, 512], F32, tag="o")
                ps_kv = psc.tile([P, 512], F32, tag="sc")
                ps_k = ps_kv[:, 0:KC]
                ps_v = ps_kv[:, KC:2 * KC]
                nco = C // P
                for co in range(nco):
                    lhsT = cur_pair[:, co, half * P:(half + 1) * P]
                    st = dict(start=(co == 0), stop=(co == nco - 1))
                    nc.tensor.matmul(ps_q0, lhsT, wq_sb[:, co, 0:512], **st)
                    nc.tensor.matmul(ps_q1, lhsT, wq_sb[:, co, 512:1024], **st)
                    nc.tensor.matmul(ps_kv, lhsT, wkv_sb[:, co, :], **st)
                if half == 0 and pr + 2 < TOKCH // 2 and (pr + 2) not in xpairs:
                    nxt = xpool.tile([P, C // P, 2 * P], BF16, tag="xt")
                    nc.sync.dma_start(nxt, xr[:, :, 2 * (pr + 2) * P:(2 * pr + 6) * P])
                    xpairs[pr + 2] = nxt
                nc.vector.tensor_copy(v_sb[:, t, :], ps_v)

                if t < TOKCH - DEFER_CHUNKS:
                    rope_part(ps_q0, 4, qT, 0, t, norm_part(ps_q0, 4, True))
                    rope_part(ps_q1, 4, qT, 4, t, norm_part(ps_q1, 4, True))
                    rope_part(ps_k, KVG, kT, 0, t, norm_part(ps_k, KVG, False))
                else:
                    # defer the rope so pre-emitted tile-0 scores keep the PE
                    # busy through this serial tail; do the norm (Sqrt) now so
                    # the act table switches to the exp set exactly once
                    rqs = (norm_part(ps_q0, 4, True), norm_part(ps_q1, 4, True),
                           norm_part(ps_k, KVG, False))
                    pending.append((ps_q0, ps_q1, ps_k, t, rqs))

            # ---- pre-emit tile-0 scores+exp for the first heads, woven
            # between the deferred rope chunks ----
            def pre_score(h, c):
                g = h // NREP
                col0 = c * P
                ps_sc = psc.tile([P, 512], F32, tag="sc")
                nc.tensor.matmul(
                    ps_sc[:, col0:512], kT[:, g, c * P:(c + 1) * P],
                    qT[:, h, col0:512], start=True, stop=True)
                pt = ppool.tile([P, 512], BF16, tag="pt")
                nc.scalar.activation(pt[:, col0:512], ps_sc[:, col0:512],
                                     AF.Exp, bias=gate[:, 0:1])
                nc.gpsimd.affine_select(
                    out=pt[:, col0:col0 + P], in_=pt[:, col0:col0 + P],
                    compare_op=mybir.AluOpType.is_ge, fill=0.0,
                    base=0, pattern=[[1, P]], channel_multiplier=-1)
                preB.setdefault(h, []).append(pt)

            # gate = 0.0, but data-dependent on the last Sqrt's input chain
            # via rq (forces every exp after every Sqrt in the Act stream:
            # exactly one table switch instead of scheduler-driven thrash)
            if pending:
                nc.vector.tensor_scalar_mul(gate, pending[-1][4][2][:, 0:1], 0.0)
            else:
                nc.vector.memset(gate, 0.0)
            for q0_, q1_, k_, tt, rqs in pending:
                rope_part(q0_, 4, qT, 0, tt, rqs[0])
                rope_part(q1_, 4, qT, 4, tt, rqs[1])
                rope_part(k_, KVG, kT, 0, tt, rqs[2])
            for h in range(PRE_HEADS):
                for c in range(4):
                    pre_score(h, c)
        actx.close()  # release the x pool before the B/C pools open

        # ======== phase B+C: attention + output projection, interleaved ====
        with tc.tile_pool(name="wpp", bufs=1) as wpool, \
             tc.tile_pool(name="pq", bufs=2) as qpool, \
             tc.tile_pool(name="sb", bufs=2) as bpool, \
             tc.tile_pool(name="so", bufs=6) as opool:
            wpr = wp.rearrange("(hc p) c -> p hc c", p=P)
            wp_ts = []
            for ct in range(C // 512):
                wp_t = wpool.tile([P, HG, 512], BF16, tag=f"wpt{ct}")
                nc.sync.dma_start(wp_t, wpr[:, :, ct * 512:(ct + 1) * 512])
                wp_ts.append(wp_t)

            def proj_unit(tk, ct):
                ps_o = pso.tile([P, 512], F32, tag="o")
                for hc in range(HG):
                    nc.tensor.matmul(
                        ps_o, yT[:, hc, tk * P:(tk + 1) * P],
                        wp_ts[ct][:, hc, :],
                        start=(hc == 0), stop=(hc == HG - 1))
                ob = opool.tile([P, 512], BF16, tag="ob")
                nc.scalar.copy(ob, ps_o)
                nc.sync.dma_start(
                    out[tk * P:(tk + 1) * P, ct * 512:(ct + 1) * 512], ob)

            cqueue = deque()
            NT = T // 512  # 4 tq tiles
            # tile-0 heads are exp-latency bound (4 chunks); alternate them
            # with tile-1 heads so the longer heads cover the bubbles
            units = [(t, h) for t in range(NT) for h in range(HG)]
            done_heads = {0: 0, 1: 0, 2: 0, 3: 0}
            for t, h in units:
                if True:
                    g = h // NREP
                    nch = 4 * (t + 1)
                    n_clean = 4 * t       # chunks in fp8 DoubleRow den pairs
                    ps_y = psy.tile([P, 512], F32, tag="y")
                    ps_sden = pss.tile([P, 512], F32, tag="sden")
                    ps_s = ps_sden[0:1, :]

                    den_plan = deque()
                    n_den_total = n_clean // 2 + (nch - n_clean)
                    den_idx = [0]
                    ptqs = {}

                    def emit_den(op_kind, arg):
                        first = den_idx[0] == 0
                        last = den_idx[0] == n_den_total - 1
                        den_idx[0] += 1
                        st = dict(start=first, stop=last)
                        if op_kind == "dr":
                            nc.tensor.matmul(
                                ps_s, ones_f8[:, :, 0:1], ptqs[arg],
                                perf_mode=mybir.MatmulPerfMode.DoubleRow,
                                skip_group_check=True, **st)
                        else:
                            c, col0, pt = arg
                            nc.tensor.matmul(ps_s[:, col0:512], ones_col,
                                             pt[:, col0:512],
                                             skip_group_check=True, **st)

                    pre = preB.get(h, None) if t == 0 else None
                    for c in range(nch):
                        o = c * P - t * 512
                        col0 = max(o, 0)
                        if pre is not None:
                            pt = pre[c]
                        else:
                            ps_sc = psc.tile([P, 512], F32, tag="sc")
                            nc.tensor.matmul(
                                ps_sc[:, col0:512], kT[:, g, c * P:(c + 1) * P],
                                qT[:, h, t * 512 + col0:(t + 1) * 512],
                                start=True, stop=True)
                            pt = ppool.tile([P, 512], BF16, tag="pt")
                            nc.scalar.activation(pt[:, col0:512],
                                                 ps_sc[:, col0:512],
                                                 AF.Exp, bias=gate[:, 0:1])
                            if o >= 0:
                                # zero the forbidden i' < j triangle post-exp
                                # on the idle gpsimd engine (exact zeros; exp
                                # no longer waits on a mask)
                                nc.gpsimd.affine_select(
                                    out=pt[:, col0:col0 + P],
                                    in_=pt[:, col0:col0 + P],
                                    compare_op=mybir.AluOpType.is_ge, fill=0.0,
                                    base=0, pattern=[[1, P]],
                                    channel_multiplier=-1)
                        st = dict(start=(c == 0), stop=(c == nch - 1))
                        nc.tensor.matmul(ps_y[:, col0:512],
                                         v_sb[:, c, g * P:(g + 1) * P],
                                         pt[:, col0:512], **st)
                        if c < n_clean:
                            prr, hf = c // 2, c % 2
                            if hf == 0:
                                ptqs[prr] = qpool.tile([P, 2, 512], F8,
                                                       tag="ptq", name="ptq")
                                nc.vector.tensor_copy(ptqs[prr][:, 0, :], pt)
                            else:
                                nc.gpsimd.tensor_copy(ptqs[prr][:, 1, :], pt)
                                den_plan.append((2 * prr + 1, "dr", prr))
                        else:
                            den_plan.append((c, "bf", (c, col0, pt)))
                        while den_plan and den_plan[0][0] <= c - DEN_DELAY:
                            _, kind, arg = den_plan.popleft()
                            emit_den(kind, arg)
                    while den_plan:
                        _, kind, arg = den_plan.popleft()
                        emit_den(kind, arg)

                    rc = bpool.tile([1, 512], F32, tag="rc")
                    nc.vector.reciprocal(rc, ps_s)
                    rb = bpool.tile([P, 512], F32, tag="rb")
                    nc.gpsimd.partition_broadcast(rb, rc)
                    nc.vector.tensor_mul(yT[:, h, t * 512:(t + 1) * 512], ps_y, rb)

                    # fill pipeline bubbles with queued projection units
                    for _ in range(2):
                        if cqueue:
                            proj_unit(*cqueue.popleft())
                    done_heads[t] += 1
                    if done_heads[t] == HG:
                        for tci in range(4):
                            for ct in range(C // 512):
                                cqueue.append((4 * t + tci, ct))
            while cqueue:
                proj_unit(*cqueue.popleft())
        ppool_ctx.close()
        psum_ctx.close()
    nc.compile()
    return nc


_NC_CACHE = []


def _get_prog():
    if not _NC_CACHE:
        _NC_CACHE.append(_build())
    return _NC_CACHE[0]


def _make_in_maps(inputs):
    x, cos, sin = inputs["x"], inputs["cos"], inputs["sin"]
    wq, wk, wv, wproj = inputs["wq"], inputs["wk"], inputs["wv"], inputs["wproj"]
    bf = ml_dtypes.bfloat16
    cos2 = np.ascontiguousarray(cos.reshape(T, D // 2), dtype=np.float32)
    sin2 = np.ascontiguousarray(sin.reshape(T, D // 2), dtype=np.float32)
    cs2 = np.ascontiguousarray(np.hstack([cos2, sin2]))
    in_maps = []
    for core in range(8):
        b, g = core // 2, core % 2
        qs = slice(g * QC, (g + 1) * QC)
        ks = slice(g * KC, (g + 1) * KC)
        in_maps.append({
            "xt": np.ascontiguousarray(x[b].T).astype(bf),
            "wq": np.ascontiguousarray(wq[:, qs]).astype(bf),
            "wkv": np.ascontiguousarray(np.hstack([wk[:, ks], wv[:, ks]])).astype(bf),
            "wp": np.ascontiguousarray(wproj[qs, :]).astype(bf),
            "csd": cs2,
        })
    return in_maps


def kernel(x, cos, sin, wq, wk, wv, wproj):
    nc = _get_prog()
    in_maps = _make_in_maps(dict(x=x, cos=cos, sin=sin, wq=wq, wk=wk, wv=wv, wproj=wproj))
    res = run_bass_kernel_spmd(nc, in_maps, core_ids=list(range(8))).results
    outp = np.empty((B, T, C), np.float32)
    for b in range(B):
        outp[b] = (res[2 * b]["out"].astype(np.float32)
                   + res[2 * b + 1]["out"].astype(np.float32))
    return outp
